# revision 26
# baseline (speedup 1.0000x reference)
"""Multi-head attention (B=4, S=2048, D=1024, H=16) on 8 TRN2 NeuronCores.

Sharding: core c handles batch b = c//2 and head-half hh = c%2 (8 heads).
Each core projects Q/K/V for only its 512 head-dim columns over all 2048
tokens, runs attention for its 8 heads, and computes a PARTIAL O
projection (its 512 input dims x full 1024 outputs). The two cores of a
batch pair are summed on the host (plus bo) — no collectives, no
duplicated projection work (17.2 GFLOP/core).

The kernel is ACT(exp)-bound (~270us of Exp at 153.6 G elem/s/core), so
the whole schedule is built to keep the PE streaming behind it without
stalls: scores for a head pair run quadrant-concurrent on PE row
halves; the PE issue order is software-pipelined (S(kc+1) lands before
PV(kc) so the exp latency is hidden); projection / O-projection matmul
units are spliced into the kc stream ~3 matmuls at a time via
generators; Q/K/V biases ride a 9th K=1 accumulation matmul (ones row x
bias row) so PSUM evacuations are plain DVE copies and the ACT queue
carries nothing but Exp; softmax normalization (ones-column rowsums,
selector-matmul partition broadcast, reciprocal+multiply) is deferred
off the critical path. All matmul operands bf16 (Wq/bq pre-scaled by
1/sqrt(head_dim)), fp32 PSUM accumulation; softmax skips
max-subtraction (scores std ~0.33, exp never overflows).
"""

import numpy as np

P = 128
D = 1024
S = 2048
HD = 512            # head-dim columns per core (8 heads x 64)
H = 8               # local heads per core
DH = 64
NCORES = 8

_STATE: dict = {}


def _build():
    from collections import deque

    import concourse.bacc as bacc
    import concourse.tile as tile
    from concourse import mybir

    f32 = mybir.dt.float32
    bf16 = mybir.dt.bfloat16
    EXP = mybir.ActivationFunctionType.Exp
    MULT = mybir.AluOpType.mult

    nc = bacc.Bacc("TRN2", target_bir_lowering=False, debug=False)

    xt = nc.dram_tensor("xt", [D, S], bf16, kind="ExternalInput").ap()
    wqt = nc.dram_tensor("wqt", [D, HD], bf16, kind="ExternalInput").ap()
    wkt = nc.dram_tensor("wkt", [D, HD], bf16, kind="ExternalInput").ap()
    wvt = nc.dram_tensor("wvt", [D, HD], bf16, kind="ExternalInput").ap()
    wot = nc.dram_tensor("wot", [HD, D], bf16, kind="ExternalInput").ap()
    bq = nc.dram_tensor("bq", [HD], f32, kind="ExternalInput").ap()
    bk = nc.dram_tensor("bk", [HD], f32, kind="ExternalInput").ap()
    bv = nc.dram_tensor("bv", [HD], f32, kind="ExternalInput").ap()
    out = nc.dram_tensor("out", [S, D], f32, kind="ExternalOutput").ap()

    with tile.TileContext(nc) as tc:
        with tc.tile_pool(name="res", bufs=1) as res, \
             tc.tile_pool(name="evac", bufs=3) as evac, \
             tc.tile_pool(name="ptp", bufs=2) as ptp, \
             tc.tile_pool(name="misc", bufs=1) as misc, \
             tc.tile_pool(name="pst", bufs=2, space="PSUM") as pstp, \
             tc.tile_pool(name="pacc", bufs=1, space="PSUM") as pacc, \
             tc.tile_pool(name="po", bufs=2, space="PSUM") as pop:

            # ---- persistent SBUF tiles ----
            xT = res.tile([P, 8, S], bf16)      # x.T        32KB/part
            qT = res.tile([P, 4, S], bf16)      # Q.T        16KB
            kT = res.tile([P, 4, S], bf16)      # K.T        16KB
            vA = res.tile([P, 16, H, DH + 1], bf16)  # V+ones 16.25KB
            aT = res.tile([P, 4, S], bf16)      # attn out.T 16KB
            wqT = res.tile([P, 8, HD], bf16)    # 8KB
            wkT = res.tile([P, 8, HD], bf16)    # 8KB
            wvT = res.tile([P, 8, HD], bf16)    # 8KB
            woT = res.tile([P, 4, D], bf16)     # 8KB
            # row sums: head h staged at partition 32*(h%4), free block h//4
            rs = res.tile([P, 2, S], f32)       # 16KB
            rsr = res.tile([P, 2, S], bf16)     # 8KB

            # ---- input DMAs: K weights + first x token-block first; the
            # sync engine issues descriptors at ~650ns each, so batch the
            # later x blocks into one DMA per token range
            nc.sync.dma_start(wkT[:], wkt.rearrange("(dc p) n -> p dc n", p=P))
            xt3 = xt.rearrange("(dc p) t -> p dc t", p=P)
            for dc in range(8):
                nc.sync.dma_start(xT[:, dc, 0:512], xt3[:, dc, 0:512])
            nc.sync.dma_start(wqT[:], wqt.rearrange("(dc p) n -> p dc n", p=P))
            br32 = misc.tile([1, 3, HD], f32)
            nc.sync.dma_start(br32[:, 0, :], bq.unsqueeze(0))
            nc.sync.dma_start(br32[:, 1, :], bk.unsqueeze(0))
            nc.sync.dma_start(br32[:, 2, :], bv.unsqueeze(0))
            brow = misc.tile([1, 3, HD], bf16)
            nc.vector.tensor_copy(brow[:], br32[:])
            ones_r = misc.tile([1, 512], bf16)
            nc.vector.memset(ones_r[:], 1.0)
            nc.sync.dma_start(wvT[:], wvt.rearrange("(dc p) n -> p dc n", p=P))
            for ts in range(1, 4):
                nc.sync.dma_start(xT[:, :, ts * 512:(ts + 1) * 512],
                                  xt3[:, :, ts * 512:(ts + 1) * 512])
            nc.sync.dma_start(woT[:], wot.rearrange("(dc p) n -> p dc n", p=P))

            nc.vector.memset(rs[:], 1.0)
            ones_c = misc.tile([P, 1], f32)
            nc.vector.memset(ones_c[:], 1.0)
            nc.vector.tensor_copy(
                vA[:, :, :, DH:DH + 1],
                ones_c[:, None, :].to_broadcast((P, 16, H, 1)))

            # selector for rowsum partition-broadcast (per pr parity)
            self32 = misc.tile([P, 2, P], f32)
            nc.vector.memset(self32[:], 0.0)
            nc.vector.memset(self32[0:1, 0, 0:64], 1.0)
            nc.vector.memset(self32[32:33, 0, 64:P], 1.0)
            nc.vector.memset(self32[64:65, 1, 0:64], 1.0)
            nc.vector.memset(self32[96:97, 1, 64:P], 1.0)
            sel = misc.tile([P, 2, P], bf16)
            nc.vector.tensor_copy(sel[:], self32[:])

            # ---- work units (generators; each yield ~= 3 matmuls) ----
            def kq_unit(wT, bias_row, dst, c, ts):
                ps = pop.tile([P, 512], f32, tag="pp", name=f"pj{c}{ts}")
                for dc in range(8):
                    nc.tensor.matmul(
                        ps[:], lhsT=wT[:, dc, c * P:(c + 1) * P],
                        rhs=xT[:, dc, ts * 512:(ts + 1) * 512],
                        start=(dc == 0), stop=False)
                    yield
                nc.tensor.matmul(ps[:], lhsT=bias_row,
                                 rhs=ones_r[:], start=False, stop=True)
                nc.vector.tensor_copy(dst[:, c, ts * 512:(ts + 1) * 512], ps[:])

            def k0_mini(kc):
                # narrow (N=128) variant of k_unit(0,0): produces kT block 0
                # keys kc*128..(kc+1)*128 so the first S-pair can start
                # ~15us earlier
                ps = pop.tile([P, 512], f32, tag="pp", name=f"km{kc}")
                for dc in range(8):
                    nc.tensor.matmul(
                        ps[:, 0:128], lhsT=wkT[:, dc, 0:P],
                        rhs=xT[:, dc, kc * P:(kc + 1) * P],
                        start=(dc == 0), stop=False)
                    yield
                nc.tensor.matmul(ps[:, 0:128],
                                 lhsT=brow[:, 1, 0:P],
                                 rhs=ones_r[:, 0:128], start=False, stop=True)
                nc.vector.tensor_copy(kT[:, 0, kc * P:(kc + 1) * P],
                                      ps[:, 0:128])

            def k_unit(c, ts):
                return kq_unit(wkT, brow[:, 1, c * P:(c + 1) * P], kT, c, ts)

            def q_unit(c, ts):
                return kq_unit(wqT, brow[:, 0, c * P:(c + 1) * P], qT, c, ts)

            def v_unit(rt):
                ps = pop.tile([P, 512], f32, tag="pp", name=f"vp{rt}")
                for dc in range(8):
                    nc.tensor.matmul(
                        ps[:], lhsT=xT[:, dc, rt * P:(rt + 1) * P],
                        rhs=wvT[:, dc, :],
                        start=(dc == 0), stop=False)
                    yield
                nc.tensor.matmul(ps[:], lhsT=ones_r[0:1, 0:128],
                                 rhs=brow[:, 2, :], start=False, stop=True)
                nc.vector.tensor_copy(
                    vA[:, rt, :, 0:DH],
                    ps.rearrange("p (h d) -> p h d", d=DH))

            def o_unit(rt, nh):
                ps = pop.tile([P, 512], f32, tag="pp", name=f"op{rt}{nh}")
                for dc in range(4):
                    nc.tensor.matmul(
                        ps[:], lhsT=aT[:, dc, rt * P:(rt + 1) * P],
                        rhs=woT[:, dc, nh * 512:(nh + 1) * 512],
                        start=(dc == 0), stop=(dc == 3))
                    yield
                ot = evac.tile([P, 512], f32, tag="outb", name=f"ot{rt}{nh}")
                nc.vector.tensor_copy(ot[:], ps[:])
                nc.sync.dma_start(
                    out[rt * P:(rt + 1) * P, nh * 512:(nh + 1) * 512], ot[:])

            def norm_unit(blk, qc):
                # one reciprocal covers both prs of the blk (their heads
                # stage on disjoint partition rows)
                qsl = slice(qc * 512, (qc + 1) * 512)
                with nc.allow_low_precision(reason="bf16 1/rowsum"):
                    nc.vector.reciprocal(rsr[:, blk, qsl], rs[:, blk, qsl])
                yield
                for pr in (2 * blk, 2 * blk + 1):
                    rb = pop.tile([P, 512], f32, tag="pp", name=f"rb{pr}{qc}")
                    nc.tensor.matmul(rb[:], lhsT=sel[:, pr % 2, :],
                                     rhs=rsr[:, blk, qsl],
                                     start=True, stop=True)
                    nc.vector.tensor_tensor(
                        aT[:, pr, qsl], aT[:, pr, qsl], rb[:], MULT)
                    yield

            def run(g):
                for _ in g:
                    pass

            # hard: (deadline_iter_idx, gen) — K/Q projection units that MUST
            # be fully issued before the iteration at that index reads them.
            # soft: norm / O-projection units with no issue deadline.
            hard = deque()
            soft = deque()

            def pump(n=1):
                while n > 0:
                    q = hard if hard else soft
                    if not q:
                        return
                    g = q[0][1] if q is hard else q[0]
                    try:
                        next(g)
                    except StopIteration:
                        q.popleft()
                        continue
                    n -= 1

            def meet_deadlines(idx):
                while hard and hard[0][0] <= idx:
                    run(hard[0][1])
                    hard.popleft()

            # ---- hard schedule: deadline in global kc-slot units ----
            # k(c,ts) first read at iter (c,0) kc=4ts; q(c,qc) at iter (c,qc)
            sched = [(kc, k0_mini(kc)) for kc in (1, 2, 3)]
            for c in range(4):
                for ts in range(4):
                    if (c, ts) != (0, 0):
                        sched.append((64 * c + 4 * ts, k_unit(c, ts)))
                for qc2 in range(4):
                    if (c, qc2) != (0, 0):
                        sched.append((64 * c + 16 * qc2, q_unit(c, qc2)))
            sched.sort(key=lambda e: e[0])
            hard.extend(sched)

            # minimal upfront prefix: attention can start after these
            run(k0_mini(0))
            run(q_unit(0, 0))

            # ---- attention, software-pipelined ----
            def s_mm(st, pr, qsl, kc):
                nc.tensor.matmul(
                    st[:, 0:512],
                    lhsT=kT[0:64, pr, kc * P:(kc + 1) * P],
                    rhs=qT[0:64, pr, qsl], start=True, stop=True,
                    tile_position=(0, 0))
                nc.tensor.matmul(
                    st[:, 512:1024],
                    lhsT=kT[64:128, pr, kc * P:(kc + 1) * P],
                    rhs=qT[64:128, pr, qsl], start=True, stop=True,
                    tile_position=(64, 0))

            # ---- flattened 256-step pipeline over (iteration, key-block) ----
            iters = [(pr, qc) for pr in range(4) for qc in range(4)]
            sts = [None, None]

            def s_issue(g):
                i, kc = divmod(g, 16)
                pr, qc = iters[i]
                qsl = slice(qc * 512, (qc + 1) * 512)
                st = pstp.tile([P, 1024], f32, tag="st", name=f"st{g}")
                s_mm(st, pr, qsl, kc)
                sts[g % 2] = st

            s_issue(0)
            run(v_unit(0))
            oaccA = oaccB = None
            for g in range(256):
                i, kc = divmod(g, 16)
                pr, qc = iters[i]
                hA, hB = 2 * pr, 2 * pr + 1
                qsl = slice(qc * 512, (qc + 1) * 512)
                if kc == 0:
                    oaccA = pacc.tile([DH + 1, 512], f32, tag="oA",
                                      name=f"oA{i}")
                    oaccB = pacc.tile([DH + 1, 512], f32, tag="oB",
                                      name=f"oB{i}")
                st = sts[g % 2]
                if g + 1 < 256:
                    meet_deadlines(g + 1)
                    s_issue(g + 1)
                if g < 15:
                    run(v_unit(g + 1))
                else:
                    pump(4 if i == 15 else (3 if (pr == 3 or not hard) else 2))
                pt = ptp.tile([P, 1024], bf16, tag="pt", name=f"pt{g}")
                nc.scalar.activation(pt[:], st[:], EXP)
                nc.tensor.matmul(
                    oaccA[:], lhsT=vA[:, kc, hA, :], rhs=pt[:, 0:512],
                    start=(kc == 0), stop=(kc == 15))
                nc.tensor.matmul(
                    oaccB[:], lhsT=vA[:, kc, hB, :], rhs=pt[:, 512:1024],
                    start=(kc == 0), stop=(kc == 15))
                if kc == 15:
                    # evacuate accumulators + rowsums (DVE), defer norm
                    nc.vector.tensor_copy(aT[0:64, pr, qsl], oaccA[0:DH, :])
                    nc.vector.tensor_copy(aT[64:128, pr, qsl],
                                          oaccB[0:DH, :])
                    nc.vector.tensor_copy(
                        rs[32 * (hA % 4):32 * (hA % 4) + 1, hA // 4, qsl],
                        oaccA[DH:DH + 1, :])
                    nc.vector.tensor_copy(
                        rs[32 * (hB % 4):32 * (hB % 4) + 1, hB // 4, qsl],
                        oaccB[DH:DH + 1, :])
                    if pr % 2 == 1:
                        soft.append(norm_unit(pr // 2, qc))
                    if pr == 3:
                        soft += [o_unit(rt, nh)
                                 for rt in range(4 * qc, 4 * qc + 4)
                                 for nh in range(2)]
            while hard or soft:
                pump(4)

    nc.compile()
    return nc


def _get_nc():
    if "nc" not in _STATE:
        _STATE["nc"] = _build()
    return _STATE["nc"]


def _make_in_maps(x, Wq, bq, Wk, bk, Wv, bv, Wo, bo):
    import ml_dtypes
    bf = ml_dtypes.bfloat16
    x = np.asarray(x, dtype=np.float32)
    scale = 1.0 / np.sqrt(DH)
    wqt = np.ascontiguousarray((np.asarray(Wq) * scale).T).astype(bf)
    wkt = np.ascontiguousarray(np.asarray(Wk).T).astype(bf)
    wvt = np.ascontiguousarray(np.asarray(Wv).T).astype(bf)
    wot = np.ascontiguousarray(np.asarray(Wo).T).astype(bf)
    bq_s = np.asarray(bq, dtype=np.float32) * scale
    bk_f = np.asarray(bk, dtype=np.float32)
    bv_f = np.asarray(bv, dtype=np.float32)
    in_maps = []
    for c in range(NCORES):
        b, hh = c // 2, c % 2
        cs = slice(hh * HD, (hh + 1) * HD)
        xtc = np.ascontiguousarray(x[b].T).astype(bf)  # [D, S]
        in_maps.append({
            "xt": xtc,
            "wqt": np.ascontiguousarray(wqt[:, cs]),
            "wkt": np.ascontiguousarray(wkt[:, cs]),
            "wvt": np.ascontiguousarray(wvt[:, cs]),
            "wot": np.ascontiguousarray(wot[cs, :]),
            "bq": np.ascontiguousarray(bq_s[cs]),
            "bk": np.ascontiguousarray(bk_f[cs]),
            "bv": np.ascontiguousarray(bv_f[cs]),
        })
    return in_maps


def kernel(x, Wq, bq, Wk, bk, Wv, bv, Wo, bo):
    from concourse.bass_utils import run_bass_kernel_spmd

    in_maps = _make_in_maps(x, Wq, bq, Wk, bk, Wv, bv, Wo, bo)
    _STATE["last_in_maps"] = in_maps
    nc = _get_nc()
    res = run_bass_kernel_spmd(nc, in_maps, list(range(NCORES)))

    B = np.asarray(x).shape[0]
    bo_f = np.asarray(bo, dtype=np.float32)
    out = np.empty((B, S, D), dtype=np.float32)
    for b in range(B):
        out[b] = res.results[2 * b]["out"] + res.results[2 * b + 1]["out"] + bo_f
    return out


# revision 28
# speedup vs baseline: 1.0252x; 1.0252x over previous
"""Multi-head attention (B=4, S=2048, D=1024, H=16) on 8 TRN2 NeuronCores.

Sharding: core c handles batch b = c//2 and head-half hh = c%2 (8 heads).
Each core projects Q/K/V for only its 512 head-dim columns over all 2048
tokens, runs attention for its 8 heads, and computes a PARTIAL O
projection (its 512 input dims x full 1024 outputs). The two cores of a
batch pair are summed on the host (plus bo) — no collectives, no
duplicated projection work (17.2 GFLOP/core).

The kernel is ACT(exp)-bound (~270us of Exp at 153.6 G elem/s/core), so
the whole schedule is built to keep the PE streaming behind it without
stalls: scores for a head pair run quadrant-concurrent on PE row
halves; the PE issue order is software-pipelined (S(kc+1) lands before
PV(kc) so the exp latency is hidden); projection / O-projection matmul
units are spliced into the kc stream ~3 matmuls at a time via
generators; Q/K/V biases ride a 9th K=1 accumulation matmul (ones row x
bias row) so PSUM evacuations are plain DVE copies and the ACT queue
carries nothing but Exp; softmax normalization (ones-column rowsums,
selector-matmul partition broadcast, reciprocal+multiply) is deferred
off the critical path. All matmul operands bf16 (Wq/bq pre-scaled by
1/sqrt(head_dim)), fp32 PSUM accumulation; softmax skips
max-subtraction (scores std ~0.33, exp never overflows).
"""

import numpy as np

P = 128
D = 1024
S = 2048
HD = 512            # head-dim columns per core (8 heads x 64)
H = 8               # local heads per core
DH = 64
NCORES = 8

_STATE: dict = {}


def _build():
    from collections import deque

    import concourse.bacc as bacc
    import concourse.tile as tile
    from concourse import mybir

    f32 = mybir.dt.float32
    bf16 = mybir.dt.bfloat16
    EXP = mybir.ActivationFunctionType.Exp
    MULT = mybir.AluOpType.mult

    nc = bacc.Bacc("TRN2", target_bir_lowering=False, debug=False)

    xt = nc.dram_tensor("xt", [D, S], bf16, kind="ExternalInput").ap()
    wqt = nc.dram_tensor("wqt", [D, HD], bf16, kind="ExternalInput").ap()
    wkt = nc.dram_tensor("wkt", [D, HD], bf16, kind="ExternalInput").ap()
    wvt = nc.dram_tensor("wvt", [D, HD], bf16, kind="ExternalInput").ap()
    wot = nc.dram_tensor("wot", [HD, D], bf16, kind="ExternalInput").ap()
    bq = nc.dram_tensor("bq", [HD], f32, kind="ExternalInput").ap()
    bk = nc.dram_tensor("bk", [HD], f32, kind="ExternalInput").ap()
    bv = nc.dram_tensor("bv", [HD], f32, kind="ExternalInput").ap()
    out = nc.dram_tensor("out", [S, D], f32, kind="ExternalOutput").ap()

    with tile.TileContext(nc) as tc:
        with tc.tile_pool(name="res", bufs=1) as res, \
             tc.tile_pool(name="evac", bufs=3) as evac, \
             tc.tile_pool(name="ptp", bufs=3) as ptp, \
             tc.tile_pool(name="misc", bufs=1) as misc, \
             tc.tile_pool(name="pst", bufs=2, space="PSUM") as pstp, \
             tc.tile_pool(name="pacc", bufs=1, space="PSUM") as pacc, \
             tc.tile_pool(name="po", bufs=2, space="PSUM") as pop:

            # ---- persistent SBUF tiles ----
            xT = res.tile([P, 8, S], bf16)      # x.T        32KB/part
            qT = res.tile([P, 4, S], bf16)      # Q.T        16KB
            kT = res.tile([P, 4, S], bf16)      # K.T        16KB
            vA = res.tile([P, 16, H, DH + 1], bf16)  # V+ones 16.25KB
            aT = res.tile([P, 4, S], bf16)      # attn out.T 16KB
            wqT = res.tile([P, 8, HD], bf16)    # 8KB
            wkT = res.tile([P, 8, HD], bf16)    # 8KB
            wvT = res.tile([P, 8, HD], bf16)    # 8KB
            woT = res.tile([P, 4, D], bf16)     # 8KB
            # row sums: head h staged at partition 32*(h%4), free block h//4
            rs = res.tile([P, 2, S], f32)       # 16KB
            rsr = res.tile([P, 2, S], bf16)     # 8KB

            # ---- input DMAs: K weights + first x token-block first; the
            # sync engine issues descriptors at ~650ns each, so batch the
            # later x blocks into one DMA per token range
            nc.sync.dma_start(wkT[:], wkt.rearrange("(dc p) n -> p dc n", p=P))
            xt3 = xt.rearrange("(dc p) t -> p dc t", p=P)
            for dc in range(8):
                nc.sync.dma_start(xT[:, dc, 0:512], xt3[:, dc, 0:512])
            nc.sync.dma_start(wqT[:], wqt.rearrange("(dc p) n -> p dc n", p=P))
            br32 = misc.tile([1, 3, HD], f32)
            nc.sync.dma_start(br32[:, 0, :], bq.unsqueeze(0))
            nc.sync.dma_start(br32[:, 1, :], bk.unsqueeze(0))
            nc.sync.dma_start(br32[:, 2, :], bv.unsqueeze(0))
            brow = misc.tile([1, 3, HD], bf16)
            nc.vector.tensor_copy(brow[:], br32[:])
            ones_r = misc.tile([1, 512], bf16)
            nc.vector.memset(ones_r[:], 1.0)
            nc.sync.dma_start(wvT[:], wvt.rearrange("(dc p) n -> p dc n", p=P))
            for ts in range(1, 4):
                nc.sync.dma_start(xT[:, :, ts * 512:(ts + 1) * 512],
                                  xt3[:, :, ts * 512:(ts + 1) * 512])
            nc.sync.dma_start(woT[:], wot.rearrange("(dc p) n -> p dc n", p=P))

            nc.vector.memset(rs[:], 1.0)
            ones_c = misc.tile([P, 1], f32)
            nc.vector.memset(ones_c[:], 1.0)
            nc.vector.tensor_copy(
                vA[:, :, :, DH:DH + 1],
                ones_c[:, None, :].to_broadcast((P, 16, H, 1)))

            # selector for rowsum partition-broadcast (per pr parity)
            self32 = misc.tile([P, 2, P], f32)
            nc.vector.memset(self32[:], 0.0)
            nc.vector.memset(self32[0:1, 0, 0:64], 1.0)
            nc.vector.memset(self32[32:33, 0, 64:P], 1.0)
            nc.vector.memset(self32[64:65, 1, 0:64], 1.0)
            nc.vector.memset(self32[96:97, 1, 64:P], 1.0)
            sel = misc.tile([P, 2, P], bf16)
            nc.vector.tensor_copy(sel[:], self32[:])

            # ---- work units (generators; each yield ~= 3 matmuls) ----
            def kq_unit(wT, bias_row, dst, c, ts):
                ps = pop.tile([P, 512], f32, tag="pp", name=f"pj{c}{ts}")
                for dc in range(8):
                    nc.tensor.matmul(
                        ps[:], lhsT=wT[:, dc, c * P:(c + 1) * P],
                        rhs=xT[:, dc, ts * 512:(ts + 1) * 512],
                        start=(dc == 0), stop=False)
                    yield
                nc.tensor.matmul(ps[:], lhsT=bias_row,
                                 rhs=ones_r[:], start=False, stop=True)
                nc.vector.tensor_copy(dst[:, c, ts * 512:(ts + 1) * 512], ps[:])

            def k0_mini(kc):
                # narrow (N=128) variant of k_unit(0,0): produces kT block 0
                # keys kc*128..(kc+1)*128 so the first S-pair can start
                # ~15us earlier
                ps = pop.tile([P, 512], f32, tag="pp", name=f"km{kc}")
                for dc in range(8):
                    nc.tensor.matmul(
                        ps[:, 0:128], lhsT=wkT[:, dc, 0:P],
                        rhs=xT[:, dc, kc * P:(kc + 1) * P],
                        start=(dc == 0), stop=False)
                    yield
                nc.tensor.matmul(ps[:, 0:128],
                                 lhsT=brow[:, 1, 0:P],
                                 rhs=ones_r[:, 0:128], start=False, stop=True)
                nc.vector.tensor_copy(kT[:, 0, kc * P:(kc + 1) * P],
                                      ps[:, 0:128])

            def k_unit(c, ts):
                return kq_unit(wkT, brow[:, 1, c * P:(c + 1) * P], kT, c, ts)

            def q_unit(c, ts):
                return kq_unit(wqT, brow[:, 0, c * P:(c + 1) * P], qT, c, ts)

            def v_unit(rt):
                ps = pop.tile([P, 512], f32, tag="pp", name=f"vp{rt}")
                for dc in range(8):
                    nc.tensor.matmul(
                        ps[:], lhsT=xT[:, dc, rt * P:(rt + 1) * P],
                        rhs=wvT[:, dc, :],
                        start=(dc == 0), stop=False)
                    yield
                nc.tensor.matmul(ps[:], lhsT=ones_r[0:1, 0:128],
                                 rhs=brow[:, 2, :], start=False, stop=True)
                nc.vector.tensor_copy(
                    vA[:, rt, :, 0:DH],
                    ps.rearrange("p (h d) -> p h d", d=DH))

            def o_unit(rt, nh):
                ps = pop.tile([P, 512], f32, tag="pp", name=f"op{rt}{nh}")
                for dc in range(4):
                    nc.tensor.matmul(
                        ps[:], lhsT=aT[:, dc, rt * P:(rt + 1) * P],
                        rhs=woT[:, dc, nh * 512:(nh + 1) * 512],
                        start=(dc == 0), stop=(dc == 3))
                    yield
                ot = evac.tile([P, 512], f32, tag="outb", name=f"ot{rt}{nh}")
                nc.vector.tensor_copy(ot[:], ps[:])
                nc.sync.dma_start(
                    out[rt * P:(rt + 1) * P, nh * 512:(nh + 1) * 512], ot[:])

            def norm_unit(blk, qc):
                # one reciprocal covers both prs of the blk (their heads
                # stage on disjoint partition rows)
                qsl = slice(qc * 512, (qc + 1) * 512)
                with nc.allow_low_precision(reason="bf16 1/rowsum"):
                    nc.vector.reciprocal(rsr[:, blk, qsl], rs[:, blk, qsl])
                yield
                for pr in (2 * blk, 2 * blk + 1):
                    rb = pop.tile([P, 512], f32, tag="pp", name=f"rb{pr}{qc}")
                    nc.tensor.matmul(rb[:], lhsT=sel[:, pr % 2, :],
                                     rhs=rsr[:, blk, qsl],
                                     start=True, stop=True)
                    nc.vector.tensor_tensor(
                        aT[:, pr, qsl], aT[:, pr, qsl], rb[:], MULT)
                    yield

            def run(g):
                for _ in g:
                    pass

            # hard: (deadline_iter_idx, gen) — K/Q projection units that MUST
            # be fully issued before the iteration at that index reads them.
            # soft: norm / O-projection units with no issue deadline.
            hard = deque()
            soft = deque()

            def pump(n=1):
                while n > 0:
                    q = hard if hard else soft
                    if not q:
                        return
                    g = q[0][1] if q is hard else q[0]
                    try:
                        next(g)
                    except StopIteration:
                        q.popleft()
                        continue
                    n -= 1

            def meet_deadlines(idx):
                while hard and hard[0][0] <= idx:
                    run(hard[0][1])
                    hard.popleft()

            # ---- hard schedule: deadline in global kc-slot units ----
            # k(c,ts) first read at iter (c,0) kc=4ts; q(c,qc) at iter (c,qc)
            sched = [(kc, k0_mini(kc)) for kc in (1, 2, 3)]
            for c in range(4):
                for ts in range(4):
                    if (c, ts) != (0, 0):
                        sched.append((64 * c + 4 * ts, k_unit(c, ts)))
                for qc2 in range(4):
                    if (c, qc2) != (0, 0):
                        sched.append((64 * c + 16 * qc2, q_unit(c, qc2)))
            sched.sort(key=lambda e: e[0])
            hard.extend(sched)

            # minimal upfront prefix: attention can start after these
            run(k0_mini(0))
            run(q_unit(0, 0))

            # ---- attention, software-pipelined ----
            def s_mm(st, pr, qsl, kc):
                nc.tensor.matmul(
                    st[:, 0:512],
                    lhsT=kT[0:64, pr, kc * P:(kc + 1) * P],
                    rhs=qT[0:64, pr, qsl], start=True, stop=True,
                    tile_position=(0, 0))
                nc.tensor.matmul(
                    st[:, 512:1024],
                    lhsT=kT[64:128, pr, kc * P:(kc + 1) * P],
                    rhs=qT[64:128, pr, qsl], start=True, stop=True,
                    tile_position=(64, 0))

            # ---- flattened 256-step pipeline over (iteration, key-block) ----
            iters = [(pr, qc) for pr in range(4) for qc in range(4)]
            sts = [None, None]

            def s_issue(g):
                i, kc = divmod(g, 16)
                pr, qc = iters[i]
                qsl = slice(qc * 512, (qc + 1) * 512)
                st = pstp.tile([P, 1024], f32, tag="st", name=f"st{g}")
                s_mm(st, pr, qsl, kc)
                sts[g % 2] = st

            s_issue(0)
            run(v_unit(0))
            oaccA = oaccB = None
            for g in range(256):
                i, kc = divmod(g, 16)
                pr, qc = iters[i]
                hA, hB = 2 * pr, 2 * pr + 1
                qsl = slice(qc * 512, (qc + 1) * 512)
                if kc == 0:
                    oaccA = pacc.tile([DH + 1, 512], f32, tag="oA",
                                      name=f"oA{i}")
                    oaccB = pacc.tile([DH + 1, 512], f32, tag="oB",
                                      name=f"oB{i}")
                st = sts[g % 2]
                if g + 1 < 256:
                    meet_deadlines(g + 1)
                    s_issue(g + 1)
                if g < 15:
                    run(v_unit(g + 1))
                    pump(1)
                else:
                    pump(4 if i == 15 else (3 if (pr == 3 or not hard) else 2))
                pt = ptp.tile([P, 1024], bf16, tag="pt", name=f"pt{g}")
                nc.scalar.activation(pt[:], st[:], EXP)
                nc.tensor.matmul(
                    oaccA[:], lhsT=vA[:, kc, hA, :], rhs=pt[:, 0:512],
                    start=(kc == 0), stop=(kc == 15))
                nc.tensor.matmul(
                    oaccB[:], lhsT=vA[:, kc, hB, :], rhs=pt[:, 512:1024],
                    start=(kc == 0), stop=(kc == 15))
                if kc == 15:
                    # evacuate accumulators + rowsums (DVE), defer norm
                    nc.vector.tensor_copy(aT[0:64, pr, qsl], oaccA[0:DH, :])
                    nc.vector.tensor_copy(aT[64:128, pr, qsl],
                                          oaccB[0:DH, :])
                    nc.vector.tensor_copy(
                        rs[32 * (hA % 4):32 * (hA % 4) + 1, hA // 4, qsl],
                        oaccA[DH:DH + 1, :])
                    nc.vector.tensor_copy(
                        rs[32 * (hB % 4):32 * (hB % 4) + 1, hB // 4, qsl],
                        oaccB[DH:DH + 1, :])
                    if pr % 2 == 1:
                        soft.append(norm_unit(pr // 2, qc))
                    if pr == 3:
                        soft += [o_unit(rt, nh)
                                 for rt in range(4 * qc, 4 * qc + 4)
                                 for nh in range(2)]
            while hard or soft:
                pump(4)

    nc.compile()
    return nc


def _get_nc():
    if "nc" not in _STATE:
        _STATE["nc"] = _build()
    return _STATE["nc"]


def _make_in_maps(x, Wq, bq, Wk, bk, Wv, bv, Wo, bo):
    import ml_dtypes
    bf = ml_dtypes.bfloat16
    x = np.asarray(x, dtype=np.float32)
    scale = 1.0 / np.sqrt(DH)
    wqt = np.ascontiguousarray((np.asarray(Wq) * scale).T).astype(bf)
    wkt = np.ascontiguousarray(np.asarray(Wk).T).astype(bf)
    wvt = np.ascontiguousarray(np.asarray(Wv).T).astype(bf)
    wot = np.ascontiguousarray(np.asarray(Wo).T).astype(bf)
    bq_s = np.asarray(bq, dtype=np.float32) * scale
    bk_f = np.asarray(bk, dtype=np.float32)
    bv_f = np.asarray(bv, dtype=np.float32)
    in_maps = []
    for c in range(NCORES):
        b, hh = c // 2, c % 2
        cs = slice(hh * HD, (hh + 1) * HD)
        xtc = np.ascontiguousarray(x[b].T).astype(bf)  # [D, S]
        in_maps.append({
            "xt": xtc,
            "wqt": np.ascontiguousarray(wqt[:, cs]),
            "wkt": np.ascontiguousarray(wkt[:, cs]),
            "wvt": np.ascontiguousarray(wvt[:, cs]),
            "wot": np.ascontiguousarray(wot[cs, :]),
            "bq": np.ascontiguousarray(bq_s[cs]),
            "bk": np.ascontiguousarray(bk_f[cs]),
            "bv": np.ascontiguousarray(bv_f[cs]),
        })
    return in_maps


def kernel(x, Wq, bq, Wk, bk, Wv, bv, Wo, bo):
    from concourse.bass_utils import run_bass_kernel_spmd

    in_maps = _make_in_maps(x, Wq, bq, Wk, bk, Wv, bv, Wo, bo)
    _STATE["last_in_maps"] = in_maps
    nc = _get_nc()
    res = run_bass_kernel_spmd(nc, in_maps, list(range(NCORES)))

    B = np.asarray(x).shape[0]
    bo_f = np.asarray(bo, dtype=np.float32)
    out = np.empty((B, S, D), dtype=np.float32)
    for b in range(B):
        out[b] = res.results[2 * b]["out"] + res.results[2 * b + 1]["out"] + bo_f
    return out
